# revision 25
# baseline (speedup 1.0000x reference)
"""Trainium2 Bass kernel for nn_AttentionHeadRankThree.

Computes, for B=4 batches:
    Q = Xq @ Wq; K = Xk @ Wk; V = Xv @ Wv          (S=4096, D_in=256, D_out=64)
    out = softmax(causal(Q K^T / sqrt(S))) @ V

Sharding: 2 cores per batch, each core owns 2048 query rows assembled from
query-block pairs {4s, 4s+3} (even cores) / {4s+1, 4s+2} (odd cores) for
s = 0..7 -- this makes the causal workload AND the program structure
identical across all 8 cores (single SPMD program).

Device-side layout: scores are computed transposed ([keys, queries] tile
orientation) so that softmax needs no max-subtraction (|logits| <~ 2 after
the 1/64 scaling) and the PV matmul consumes exp(scores) directly as the
moving operand.  The softmax denominator falls out of an appended
ones-column on V.  Matmul operands are bf16 (PSUM accumulation and all
softmax arithmetic stay fp32; the denominator uses the same rounded E as
the numerator, so softmax normalization is exact w.r.t. operand rounding).
Host does only layout work (transpose / gather / dtype cast).
"""

import sys

sys.path.insert(0, "/opt/trn_rl_repo")

import ml_dtypes
import numpy as np

import concourse.bass as bass
import concourse.bacc as bacc
import concourse.mybir as mybir
import concourse.tile as tile
from concourse.bass_utils import run_bass_kernel_spmd

B, S, DI, DO = 4, 4096, 256, 64
NCORES = 8
W = 256          # query columns per slot
NSLOT = 8        # slots per core -> NSLOT*W = 2048 query rows per core
NKB = 32         # 128-row key blocks per batch
GRP = 4          # key blocks per psum/exp group
P = 128
F32 = mybir.dt.float32
BF16 = mybir.dt.bfloat16
SCALE = 1.0 / 64.0          # reference scales by sqrt(window)=sqrt(4096)=64
MASK_W = [128, 128, 256, 256]  # masked width at diag position r=0..3
ts = bass.ts
BF = ml_dtypes.bfloat16


def blocks_for(side):
    out = []
    for s in range(NSLOT):
        out += [4 * s, 4 * s + 3] if side == 0 else [4 * s + 1, 4 * s + 2]
    return out


def masks_for(side):
    kk = np.arange(128)[:, None]
    qq = np.arange(128)[None, :]
    tri = (kk <= qq).astype(np.float32)
    ones = np.ones((128, 128), np.float32)
    zer = np.zeros((128, 128), np.float32)
    d0, d1 = (0, 3) if side == 0 else (1, 2)

    def cell(r, d):
        return ones if r < d else (tri if r == d else zer)

    m = np.zeros((4, 128, W), np.float32)
    for r in range(4):
        m[r, :, :128] = cell(r, d0)
        m[r, :, 128:] = cell(r, d1)
    return m.astype(BF)


def slice_of(i):
    # group-local psum slice: evens first, then odds; even/odd pairs
    # (i, i+1) land in distinct PSUM banks of the [128, 1024] group tile.
    return i // 2 if i % 2 == 0 else GRP // 2 + i // 2


def kernel_body(nc, tc, d):
    Exp = mybir.ActivationFunctionType.Exp
    with (
        tc.tile_pool(name="persist", bufs=1) as persist,
        tc.tile_pool(name="E", bufs=15) as epool,
        tc.tile_pool(name="small", bufs=2) as spool,
        tc.tile_pool(name="qkps", bufs=3, space="PSUM") as qkps,
        tc.tile_pool(name="pvps", bufs=2, space="PSUM") as pvps,
    ):
        # ---------------- persistent SBUF ----------------
        xq_sb = persist.tile([P, 2, NSLOT * W], BF16, tag="xq")
        xk_sb = persist.tile([P, 2, S], BF16, tag="xk")
        xv_sb = persist.tile([P, 2, S], BF16, tag="xv")
        # packed consts: wqd[256] | wkd[256] | wv[128] | masks[1024]
        const_sb = persist.tile([P, 1664], BF16, tag="const")
        wqd_sb = const_sb[:, 0:256].rearrange("p (c x) -> p c x", c=2)
        wkd_sb = const_sb[:, 256:512].rearrange("p (c x) -> p c x", c=2)
        wv_sb = const_sb[:, 512:640].rearrange("p (c x) -> p c x", c=2)
        mask_sb = const_sb[:, 640:1664].rearrange("p (r x) -> p r x", r=4)
        id_sb = persist.tile([P, 128], F32, tag="id")
        # Q^T with rows 0:64 = Q^T and 64:128 = duplicate (for PE row-tiling)
        QT_sb = persist.tile([P, NSLOT * W], BF16, tag="QT")
        # K^T: partitions 0:64 hold even key-blocks, 64:128 odd key-blocks
        KT_sb = persist.tile([P, NKB // 2 * 128], BF16, tag="KT")
        V_sb = persist.tile([P, NKB, DO + 1], BF16, tag="V")  # col 64 = ones

        # ---------------- input DMAs ----------------
        # One DMA covers both c-chunks of a column range (3D AP), so the
        # scalar queue needs only 9 trigger instructions ahead of the exps.
        # xv rides the sync queue: its need-by times (PV, trailing the ACT
        # exp pacer) leave ~20 us of slack.
        ones_sb = persist.tile([P, NKB], F32, tag="ones")
        nc.vector.memset(ones_sb[:], 1.0)
        nc.vector.tensor_copy(
            V_sb[:, :, DO : DO + 1].rearrange("p k one -> p (k one)"), ones_sb[:]
        )
        xqr = d["xqT"].rearrange("(c p) s -> p c s", p=P)
        xkr = d["xkT"].rearrange("(c p) s -> p c s", p=P)
        xvr = d["xvT"].rearrange("(c p) s -> p c s", p=P)

        # xv rides the gpsimd SWDGE queue (3rd queue; input loads only --
        # DRAM-destination SWDGE wedges the device, DRAM-source is the
        # documented pattern).  That frees sync to deliver xq_j0 early.
        for j in range(4):
            nc.gpsimd.dma_start(xv_sb[:, :, ts(j, 1024)], xvr[:, :, ts(j, 1024)])
        nc.sync.dma_start(xq_sb[:, :, 0:1024], xqr[:, :, 0:1024])
        nc.scalar.dma_start(const_sb[:, 0:640], d["constb"][:, 0:640])  # weights
        nc.scalar.dma_start(xk_sb[:, :, 0:1024], xkr[:, :, 0:1024])
        nc.scalar.dma_start(const_sb[:, 640:1664], d["constb"][:, 640:1664])
        nc.scalar.dma_start(xq_sb[:, :, 1024:2048], xqr[:, :, 1024:2048])
        nc.scalar.dma_start(id_sb[:], d["ident"][:])
        # xk pieces j=1..3 are emitted just-in-time inside the slot loop so
        # their triggers interleave between exps in the ACT stream instead of
        # blocking ahead of them on HWDGE queue space.

        # ---------------- projection emitters ----------------
        def q_proj(p4):
            ps = qkps.tile([P, 512], F32, tag="qk")
            for c in range(2):
                nc.tensor.matmul(
                    ps[:],
                    wqd_sb[:, c, :],
                    xq_sb[:, c, ts(p4, 512)],
                    start=(c == 0),
                    stop=(c == 1),
                )
            nc.vector.tensor_copy(QT_sb[:, ts(p4, 512)], ps[:])

        def k_proj(p8):
            # K^T -> split even/odd key-blocks into top/bottom partition halves
            ps = qkps.tile([P, 512], F32, tag="qk")
            for c in range(2):
                nc.tensor.matmul(
                    ps[:],
                    wkd_sb[:, c, :],
                    xk_sb[:, c, ts(p8, 512)],
                    start=(c == 0),
                    stop=(c == 1),
                )
            src = ps.rearrange("p (g t q) -> p g t q", t=2, q=128)
            dst = KT_sb[:, ts(p8, 256)].rearrange("p (g q) -> p g q", q=128)
            nc.vector.tensor_copy(dst[0:64], src[0:64, :, 0, :])
            nc.vector.tensor_copy(dst[64:128], src[64:128, :, 1, :])

        def v_proj(vp):
            # V natural [s, d] (+ ones col added above)
            ps = qkps.tile([P, 4, DO], F32, tag="qk")
            for j in range(4):
                sblk = 4 * vp + j
                for c in range(2):
                    nc.tensor.matmul(
                        ps[:, j, :],
                        xv_sb[:, c, ts(sblk, 128)],
                        wv_sb[:, c, :],
                        start=(c == 0),
                        stop=(c == 1),
                    )
            nc.vector.tensor_copy(V_sb[:, 4 * vp : 4 * vp + 4, 0:DO], ps[:])

        def qk_group(s, a):
            # one [GRP key-blocks x W queries] score group + exp
            ps = qkps.tile([P, GRP * W], F32, tag="qk")
            E = epool.tile([P, GRP * W], BF16, tag="E")
            for i in range(GRP):
                kb = a + i
                lo, hi = (0, 64) if kb % 2 == 0 else (64, 128)
                nc.tensor.matmul(
                    ps[:, ts(slice_of(i), W)],
                    KT_sb[lo:hi, ts(kb // 2, 128)],
                    QT_sb[lo:hi, ts(s, W)],
                    start=True,
                    stop=True,
                )
            nc.scalar.activation(E[:], ps[:], Exp, scale=SCALE)
            return E

        def qk_masks(s, E_last):
            for i in range(GRP):
                w = MASK_W[i]
                c0 = slice_of(i) * W
                nc.vector.tensor_mul(
                    E_last[:, c0 : c0 + w], E_last[:, c0 : c0 + w],
                    mask_sb[:, i, 0:w],
                )

        def pv_group(s, g, a, pv, Es):
            n_kb = 4 * (s + 1)
            for i in range(GRP):
                kb = a + i
                nc.tensor.matmul(
                    pv[:],
                    V_sb[:, kb, :],
                    Es[g][:, ts(slice_of(i), W)],
                    start=(kb == 0),
                    stop=(kb == n_kb - 1),
                    skip_group_check=True,
                )

        def out_stage(s, pv):
            pv_sb = spool.tile([DO + 1, W], F32, tag="pvsb")
            nc.vector.tensor_copy(pv_sb[:], pv[:])
            for j in range(2):
                tp = qkps.tile([P, DO + 1], F32, tag="qk")
                nc.tensor.transpose(
                    tp[:], pv_sb[:, ts(j, 128)], id_sb[0 : DO + 1, 0 : DO + 1]
                )
                rc = spool.tile([P, 1], F32, tag="rc")
                nc.vector.reciprocal(rc[:], tp[:, DO : DO + 1])
                ob = spool.tile([P, DO], F32, tag="ob")
                nc.vector.tensor_scalar_mul(ob[:], tp[:, 0:DO], rc[:])
                nc.sync.dma_start(d["out"][ts(2 * s + j, 128), :], ob[:])

        # ---------------- attention (software-pipelined emission) ---------
        # Slot s+1's QK/exp stage is emitted BEFORE slot s's PV stage so the
        # ACT engine never starves behind PV work; E buffers span two slots.
        prev = None  # (s, Es, pv)
        for s in range(NSLOT):
            if s in (1, 3, 5):
                j = (s + 1) // 2
                nc.scalar.dma_start(
                    xk_sb[:, :, ts(j, 1024)], xkr[:, :, ts(j, 1024)]
                )
            if s % 2 == 0:
                q_proj(s // 2)
            k_proj(s)
            v_proj(s)
            Es = [qk_group(s, a) for a in range(0, 4 * (s + 1), GRP)]
            qk_masks(s, Es[-1])
            pv = pvps.tile([DO + 1, W], F32, tag="pv")
            if prev is not None:
                ps_, pEs_, ppv_ = prev
                for g in range(len(pEs_)):
                    pv_group(ps_, g, g * GRP, ppv_, pEs_)
                out_stage(ps_, ppv_)
            prev = (s, Es, pv)
        ps_, pEs_, ppv_ = prev
        for g in range(len(pEs_)):
            pv_group(ps_, g, g * GRP, ppv_, pEs_)
        out_stage(ps_, ppv_)


_PROGRAM = None


def build_program():
    global _PROGRAM
    if _PROGRAM is not None:
        return _PROGRAM
    nc = bacc.Bacc(
        "TRN2", target_bir_lowering=False, debug=False, num_devices=NCORES
    )
    d = {}
    for name, shape in [
        ("xqT", [DI, NSLOT * W]),
        ("xkT", [DI, S]),
        ("xvT", [DI, S]),
        ("constb", [128, 1664]),
    ]:
        d[name] = nc.dram_tensor(name, shape, BF16, kind="ExternalInput").ap()
    d["ident"] = nc.dram_tensor("ident", [128, 128], F32, kind="ExternalInput").ap()
    d["out"] = nc.dram_tensor("out", [NSLOT * W, DO], F32, kind="ExternalOutput").ap()
    with tile.TileContext(nc) as tc:
        kernel_body(nc, tc, d)
    nc.compile()
    _PROGRAM = (nc, d)
    return _PROGRAM


def shard_inputs(inputs):
    xq = np.asarray(inputs["inputs_for_queries"], np.float32)
    xk = np.asarray(inputs["inputs_for_keys"], np.float32)
    xv = np.asarray(inputs["inputs_for_values"], np.float32)
    wq = np.asarray(inputs["q_weight"], np.float32).astype(BF)
    wk = np.asarray(inputs["k_weight"], np.float32).astype(BF)
    wv = np.asarray(inputs["v_weight"], np.float32).astype(BF)
    ident = np.eye(128, dtype=np.float32)

    def dup(w):  # [256, 64] -> [128, 2, 128] duplicated-col chunks -> [128, 256]
        return np.concatenate(
            [np.concatenate([w[c * 128 : (c + 1) * 128]] * 2, axis=1) for c in (0, 1)],
            axis=1,
        )

    wvp = np.concatenate([wv[0:128], wv[128:256]], axis=1)  # [128, 128]
    constb = [
        np.concatenate(
            [dup(wq), dup(wk), wvp, m.transpose(1, 0, 2).reshape(128, 1024)], axis=1
        ).astype(BF)
        for m in (masks_for(0), masks_for(1))
    ]
    in_maps = []
    for c in range(NCORES):
        b, side = c // 2, c % 2
        rows = np.concatenate(
            [np.arange(128 * g, 128 * g + 128) for g in blocks_for(side)]
        )
        in_maps.append(
            {
                "xqT": np.ascontiguousarray(xq[b][rows].T).astype(BF),
                "xkT": np.ascontiguousarray(xk[b].T).astype(BF),
                "xvT": np.ascontiguousarray(xv[b].T).astype(BF),
                "constb": constb[side],
                "ident": ident,
            }
        )
    return in_maps


def unshard(outs):
    full = np.empty((B, S, DO), np.float32)
    for c in range(NCORES):
        b, side = c // 2, c % 2
        for i, g in enumerate(blocks_for(side)):
            full[b, 128 * g : 128 * g + 128] = outs[c][128 * i : 128 * i + 128]
    return full


def run(inputs, **spmd_kwargs):
    nc, _ = build_program()
    in_maps = shard_inputs(inputs)
    res = run_bass_kernel_spmd(
        nc, in_maps, core_ids=list(range(NCORES)), **spmd_kwargs
    )
    return unshard([r["out"] for r in res.results]), res


def kernel(**inputs):
    out, _ = run(inputs)
    return out
